# revision 22
# baseline (speedup 1.0000x reference)
import sys, os
sys.path.insert(0, '/opt/trn_rl_repo')
import numpy as np

import concourse.bass as bass
import concourse.tile as tile
import concourse.bacc as bacc
from concourse import mybir
from concourse.bass_utils import run_bass_kernel_spmd

F32 = mybir.dt.float32
AF = mybir.ActivationFunctionType
ALU = mybir.AluOpType

# problem constants (hardcoded per contract)
NCORES = 8
HID = 128
MAXLEN = 1001
B = 128                     # genes
N = B * MAXLEN              # 128128 nodes
E = 512000
LC = 79                     # L_CNN
GPC = B // NCORES           # 16 genes per core
NS = N // NCORES            # 16016 nodes per core
NTILE = 126                 # dest tiles of 128 (125 full + 1x16)
NSP = NTILE * 128           # 16128 padded
BN_EPS = 1e-5

_cache = {}


def _prep(x, edge_index, edge_attr, params):
    """Host-side index/layout preprocessing. Returns (in_maps, K_c)."""
    f32 = np.float32
    xv = np.asarray(x).astype(np.int64)
    row = np.asarray(edge_index[0]).astype(np.int64)
    col = np.asarray(edge_index[1]).astype(np.int64)
    ea = np.asarray(edge_attr).astype(f32)

    deg = np.bincount(col, weights=ea.astype(np.float64), minlength=N)
    dinv = np.zeros(N, np.float64)
    nz = deg > 0
    dinv[nz] = 1.0 / np.sqrt(deg[nz])
    norm = (dinv[row] * ea * dinv[col]).astype(f32)

    # CX encoding (11, N): [C(5); onehot5(x)(5); ones]
    Cm = np.bincount(col * 5 + xv[row], weights=norm.astype(np.float64),
                     minlength=5 * N).astype(f32).reshape(N, 5).T
    X5 = (xv[None, :] == np.arange(5)[:, None]).astype(f32)
    CX = np.concatenate([Cm, X5, np.ones((1, N), f32)], 0)     # (11, N)

    # --- edges sorted by destination, grouped per (core, tile) ---
    order = np.argsort(col, kind='stable')
    col_s, row_s, norm_s = col[order], row[order], norm[order]
    core_of = col_s // NS
    loc = col_s - core_of * NS
    tile_of = loc // 128
    slot = (loc - tile_of * 128).astype(f32)

    node_lo = (np.arange(NCORES)[:, None] * NS +
               np.arange(NTILE)[None, :] * 128).ravel()        # (8*126,)
    starts = np.searchsorted(col_s, node_lo)
    grp = (core_of * NTILE + tile_of).astype(np.int64)
    rank = np.arange(E) - starts[grp]
    counts = np.bincount(grp, minlength=NCORES * NTILE)
    K_c = int(np.ceil(counts.max() / 128))
    NCH = NTILE * K_c
    EPC = NCH * 128

    gpos = grp * (K_c * 128) - core_of * (NTILE * K_c * 128 - EPC)  # == grp*K_c*128
    gpos = grp * K_c * 128 + rank                                   # global padded slot
    cxe = np.zeros((11, NCORES * EPC), f32)
    cxe[:, gpos] = CX[:, row_s]
    norm_p = np.zeros(NCORES * EPC, f32)
    norm_p[gpos] = norm_s
    slot_p = np.zeros(NCORES * EPC, f32)
    slot_p[gpos] = slot

    # --- params ---
    p = {k: (v if isinstance(v, dict) else np.asarray(v, f32)) for k, v in params.items()}
    def A(v): return np.ascontiguousarray(np.asarray(v, np.float32))
    emb = A(p['emb'])
    perm = [0, 1, 3, 2]   # gate reorder i,f,g,o -> i,f,o,g

    def reord(w):         # (4H, ...) -> block-permuted
        wb = w.reshape(4, HID, -1)
        return wb[perm].reshape(4 * HID, -1)

    gl = p['gene_lstm']
    wih_gene_T = A(reord(A(gl['Wih'])).T)                      # (256, 512)
    whh_gene_T = A(reord(A(gl['Whh'])).T)                      # (128, 512)
    bias_gene = A((reord((A(gl['bih']) + A(gl['bhh']))[:, None]))
                  .reshape(4, HID).T)                          # (128, 4)

    tops = {}
    for d, nm in (('f', 'top_fwd'), ('b', 'top_bwd')):
        tp = p[nm]
        tops['wih_' + d] = A(reord(A(tp['Wih'])))              # (512, 20224)
        tops['whh_' + d + '_t'] = A(reord(A(tp['Whh'])).T)     # (128, 512)
        tops['bias_' + d] = A(reord((A(tp['bih']) + A(tp['bhh']))[:, None])
                              .reshape(4, HID).T)              # (128, 4)

    def bn_prep(convp, bnp):
        sc = A(bnp['gamma']) / np.sqrt(1.0 + BN_EPS)
        bi = A(convp['b']) * sc + A(bnp['beta'])
        return sc.reshape(HID, 1), bi.reshape(HID, 1)

    bn1s, bn1b = bn_prep(p['conv1'], p['bn1'])
    bn2s, bn2b = bn_prep(p['conv2'], p['bn2'])

    waug1 = np.concatenate([
        emb @ A(p['arma1']['Wi']),
        emb @ A(p['arma1']['Wr']),
        A(p['arma1']['b']).reshape(1, HID)], 0)                # (11, 128)
    shared = {
        'waug1': A(waug1),
        'wi2': A(p['arma2']['Wi']), 'wr2': A(p['arma2']['Wr']),
        'b2col': A(p['arma2']['b']).reshape(HID, 1),
        'conv1wt': A(A(p['conv1']['w']).transpose(1, 2, 0)),   # (I, 12, O)
        'conv2wt': A(A(p['conv2']['w']).transpose(1, 2, 0)),
        'bn1s': bn1s, 'bn1b': bn1b, 'bn2s': bn2s, 'bn2b': bn2b,
        'wihgT': wih_gene_T, 'whhgT': whh_gene_T, 'biasg': bias_gene,
        'whhfT': tops['whh_f_t'], 'whhbT': tops['whh_b_t'],
        'biasf': tops['bias_f'], 'biasb': tops['bias_b'],
        'w1tA': A(A(p['ffn']['W1'])[:, :HID].T),               # (128, 32)
        'w1tB': A(A(p['ffn']['W1'])[:, HID:].T),
        'b1col': A(p['ffn']['b1']).reshape(32, 1),
        'w2t': A(A(p['ffn']['W2']).T),                         # (32, 1)
        'iota': np.tile(np.arange(128, dtype=f32), (128, 1)),
        'ident': np.eye(128, dtype=f32),
    }

    in_maps = []
    for c in range(NCORES):
        cx_s = np.zeros((11, NSP), f32)
        cx_s[:, :NS] = CX[:, c * NS:(c + 1) * NS]
        sl = slice(c * EPC, (c + 1) * EPC)
        if c < 4:
            shard = tops['wih_f'][c * 128:(c + 1) * 128]       # (128, 20224)
        else:
            shard = tops['wih_b'][(c - 4) * 128:(c - 3) * 128]
        m = dict(shared)
        m.update({
            'cxts': cx_s,
            'cxe': A(cxe[:, sl]),                              # (11, EPC)
            'norme': A(norm_p[sl].reshape(NCH, 128).T),        # (128, NCH)
            'dslot': A(slot_p[sl].reshape(NCH, 128).T),        # (128, NCH)
            'wihshard': A(shard.T.reshape(158, 128, 128)),     # (158,128,128)
        })
        in_maps.append(m)
    return in_maps, K_c


def _build(K_c):
    NCH = NTILE * K_c
    EPC = NCH * 128
    nc = bacc.Bacc("TRN2", target_bir_lowering=False, debug=False,
                   num_devices=NCORES)
    D = {}
    def din(name, shape):
        D[name] = nc.dram_tensor(name, list(shape), F32, kind="ExternalInput").ap()
    for name, shape in [
        ('cxts', (11, NSP)), ('cxe', (11, EPC)), ('norme', (128, NCH)),
        ('dslot', (128, NCH)), ('waug1', (11, 128)),
        ('wi2', (128, 128)), ('wr2', (128, 128)), ('b2col', (128, 1)),
        ('conv1wt', (128, 12, 128)), ('conv2wt', (128, 12, 128)),
        ('bn1s', (128, 1)), ('bn1b', (128, 1)), ('bn2s', (128, 1)), ('bn2b', (128, 1)),
        ('wihgT', (256, 512)), ('whhgT', (128, 512)), ('biasg', (128, 4)),
        ('wihshard', (158, 128, 128)),
        ('whhfT', (128, 512)), ('whhbT', (128, 512)),
        ('biasf', (128, 4)), ('biasb', (128, 4)),
        ('w1tA', (128, 32)), ('w1tB', (128, 32)), ('b1col', (32, 1)),
        ('w2t', (32, 1)), ('iota', (128, 128)), ('ident', (128, 128)),
    ]:
        din(name, shape)
    out_d = nc.dram_tensor("out", [1, B], F32, kind="ExternalOutput").ap()

    with tile.TileContext(nc) as tc:
        # ---------- persistent pools ----------
        with tc.tile_pool(name="consts", bufs=1) as consts, \
             tc.tile_pool(name="seq", bufs=1) as seqp, \
             tc.tile_pool(name="dram", bufs=1, space="DRAM") as dram:

            def ld(name, shape):
                t = consts.tile(list(shape), F32, name=name + "_sb")
                nc.gpsimd.dma_start(t[:], D[name][:])
                return t

            ident = ld('ident', (128, 128))
            iota = ld('iota', (128, 128))
            wi2 = ld('wi2', (128, 128)); wr2 = ld('wr2', (128, 128))
            b2col = ld('b2col', (128, 1))
            waug1 = ld('waug1', (11, 128))

            # residents across phases
            hT = seqp.tile([128, LC, GPC], F32)     # CNN out, feat x pos x gene
            hgT = seqp.tile([128, LC, GPC], F32)    # gene-lstm out (relu)
            bT = seqp.tile([128, LC, GPC], F32)     # attention out

            # ================= ARMA =================
            with tc.tile_pool(name="f2pool", bufs=1) as f2pool:
                F2 = f2pool.tile([128, NSP], F32)

                with tc.tile_pool(name="arma", bufs=1) as arma, \
                     tc.tile_pool(name="estream", bufs=3) as estream, \
                     tc.tile_pool(name="cxstream", bufs=3) as cxstream, \
                     tc.tile_pool(name="awork", bufs=4) as awork, \
                     tc.tile_pool(name="psA", bufs=2, space=bass.MemorySpace.PSUM) as psA, \
                     tc.tile_pool(name="psG", bufs=2, space=bass.MemorySpace.PSUM) as psG, \
                     tc.tile_pool(name="psB", bufs=2, space=bass.MemorySpace.PSUM) as psB:

                    norme = arma.tile([128, NCH], F32)
                    nc.gpsimd.dma_start(norme[:], D['norme'][:])
                    dslot = arma.tile([128, NCH], F32)
                    nc.gpsimd.dma_start(dslot[:], D['dslot'][:])

                    F1 = arma.tile([128, NSP], F32)

                    # F1 shard: relu(waug1.T @ cxts)
                    for t in range(NTILE):
                        cxt = cxstream.tile([11, 128], F32)
                        nc.gpsimd.dma_start(cxt[:], D['cxts'][:, bass.ts(t, 128)])
                        ps = psA.tile([128, 128], F32)
                        nc.tensor.matmul(ps[:], waug1[:], cxt[:])
                        nc.scalar.activation(F1[:, bass.ts(t, 128)], ps[:], AF.Relu)

                    # layer-2 aggregation + dense
                    for t in range(NTILE):
                        ecx = estream.tile([11, K_c * 128], F32)
                        nc.gpsimd.dma_start(ecx[:], D['cxe'][:, bass.ts(t, K_c * 128)])
                        gt = psG.tile([128, 128], F32)
                        for j in range(K_c):
                            ch = t * K_c + j
                            f1e_ps = psA.tile([128, 128], F32)
                            nc.tensor.matmul(f1e_ps[:], ecx[:, bass.ts(j, 128)], waug1[:])
                            f1e = awork.tile([128, 128], F32)
                            nc.scalar.activation(f1e[:], f1e_ps[:], AF.Relu,
                                                 scale=norme[:, ch:ch + 1])
                            oh = awork.tile([128, 128], F32)
                            nc.vector.tensor_scalar(oh[:], iota[:],
                                                    dslot[:, ch:ch + 1], None,
                                                    ALU.is_equal)
                            nc.tensor.matmul(gt[:], f1e[:], oh[:],
                                             start=(j == 0), stop=(j == K_c - 1))
                        gts = awork.tile([128, 128], F32)
                        nc.vector.tensor_copy(gts[:], gt[:])
                        f2ps = psB.tile([128, 128], F32)
                        nc.tensor.matmul(f2ps[:], wi2[:], gts[:], start=True, stop=False)
                        nc.tensor.matmul(f2ps[:], wr2[:], F1[:, bass.ts(t, 128)],
                                         start=False, stop=True)
                        nc.scalar.activation(F2[:, bass.ts(t, 128)], f2ps[:], AF.Relu,
                                             bias=b2col[:, 0:1])

                # ================= CNN ================= (uses F2, inside f2pool scope)
                with tc.tile_pool(name="cnnw", bufs=1) as cnnw, \
                     tc.tile_pool(name="cwork", bufs=2) as cwork, \
                     tc.tile_pool(name="psC", bufs=2, space=bass.MemorySpace.PSUM) as psC:
                    w1t = cnnw.tile([128, 12, 128], F32)
                    nc.gpsimd.dma_start(w1t[:], D['conv1wt'][:])
                    w2t_c = cnnw.tile([128, 12, 128], F32)
                    nc.gpsimd.dma_start(w2t_c[:], D['conv2wt'][:])
                    bn1s = ld('bn1s', (128, 1)); bn1b = ld('bn1b', (128, 1))
                    bn2s = ld('bn2s', (128, 1)); bn2b = ld('bn2b', (128, 1))

                    for g in range(GPC):
                        xg = F2[:, g * MAXLEN: g * MAXLEN + MAXLEN]
                        c1 = cwork.tile([128, 330, 3], F32)
                        c1f = c1[:].rearrange('p a b -> p (a b)')
                        for lo, w in ((0, 512), (512, 478)):
                            cps = psC.tile([128, w], F32)
                            for k in range(12):
                                nc.tensor.matmul(cps[:], w1t[:, k, :],
                                                 xg[:, lo + k: lo + k + w],
                                                 start=(k == 0), stop=(k == 11))
                            nc.scalar.activation(c1f[:, lo:lo + w], cps[:], AF.Lrelu,
                                                 scale=bn1s[:, 0:1], bias=bn1b[:, 0:1],
                                                 alpha=0.01)
                        p1 = cwork.tile([128, 330], F32)
                        nc.vector.tensor_max(p1[:], c1[:, :, 0], c1[:, :, 1])
                        nc.vector.tensor_max(p1[:], p1[:], c1[:, :, 2])
                        c2 = cwork.tile([128, 80, 4], F32)
                        c2f = c2[:].rearrange('p a b -> p (a b)')
                        cps2 = psC.tile([128, 319], F32)
                        for k in range(12):
                            nc.tensor.matmul(cps2[:], w2t_c[:, k, :], p1[:, k:k + 319],
                                             start=(k == 0), stop=(k == 11))
                        nc.scalar.activation(c2f[:, 0:319], cps2[:], AF.Lrelu,
                                             scale=bn2s[:, 0:1], bias=bn2b[:, 0:1],
                                             alpha=0.01)
                        p2 = cwork.tile([128, LC], F32)
                        nc.vector.tensor_max(p2[:], c2[:, 0:LC, 0], c2[:, 0:LC, 1])
                        nc.vector.tensor_max(p2[:], p2[:], c2[:, 0:LC, 2])
                        nc.vector.tensor_max(hT[:, :, g], p2[:], c2[:, 0:LC, 3])

            # ================= leave-one-out all-reduce =================
            Tpart = seqp.tile([128, LC], F32)
            nc.vector.tensor_reduce(Tpart[:], hT[:], mybir.AxisListType.X, ALU.add)
            t_in = dram.tile([128, LC], F32)
            t_out = dram.tile([128, LC], F32)
            nc.gpsimd.dma_start(t_in[:], Tpart[:])
            nc.gpsimd.collective_compute(
                "AllReduce", ALU.add, replica_groups=[list(range(NCORES))],
                ins=[t_in.opt()], outs=[t_out.opt()])
            Tall = seqp.tile([128, LC], F32)
            nc.gpsimd.dma_start(Tall[:], t_out[:])

            # ================= gene LSTM =================
            with tc.tile_pool(name="glstm", bufs=1) as glstm, \
                 tc.tile_pool(name="gwork", bufs=4) as gwork, \
                 tc.tile_pool(name="psD", bufs=2, space=bass.MemorySpace.PSUM) as psD, \
                 tc.tile_pool(name="psE", bufs=2, space=bass.MemorySpace.PSUM) as psE:
                NTs = glstm.tile([128, LC, GPC], F32)
                # NT = Tall (broadcast over gene) - h
                nc.vector.scalar_tensor_tensor(
                    NTs[:], hT[:], -1.0,
                    Tall[:].unsqueeze(-1).broadcast_to((128, LC, GPC)),
                    ALU.mult, ALU.add)
                wihgA = glstm.tile([128, 512], F32)
                nc.gpsimd.dma_start(wihgA[:], D['wihgT'][0:128, :])
                wihgB = glstm.tile([128, 512], F32)
                nc.gpsimd.dma_start(wihgB[:], D['wihgT'][128:256, :])
                whhgT = glstm.tile([128, 512], F32)
                nc.gpsimd.dma_start(whhgT[:], D['whhgT'][:])
                biasg = glstm.tile([128, 4], F32)
                nc.gpsimd.dma_start(biasg[:], D['biasg'][:])

                xps = glstm.tile([128, 4, LC * GPC], F32)
                for gate in range(4):
                    for lo, w in ((0, 512), (512, 512), (1024, 240)):
                        ps = psD.tile([128, w], F32)
                        nc.tensor.matmul(ps[:], wihgA[:, bass.ts(gate, 128)],
                                         hT[:].rearrange('p a b -> p (a b)')[:, lo:lo + w],
                                         start=True, stop=False)
                        nc.tensor.matmul(ps[:], wihgB[:, bass.ts(gate, 128)],
                                         NTs[:].rearrange('p a b -> p (a b)')[:, lo:lo + w],
                                         start=False, stop=True)
                        nc.scalar.activation(xps[:, gate, lo:lo + w], ps[:],
                                             AF.Identity, bias=biasg[:, gate:gate + 1])

                hstate = glstm.tile([128, GPC], F32)
                cstate = glstm.tile([128, GPC], F32)
                nc.vector.memset(hstate[:], 0.0)
                nc.vector.memset(cstate[:], 0.0)
                xps_v = xps[:]  # (128, 4, 79*16)
                for l in range(LC):
                    gps = psE.tile([128, 4, GPC], F32)
                    for gate in range(4):
                        nc.tensor.matmul(gps[:, gate, :],
                                         whhgT[:, bass.ts(gate, 128)], hstate[:])
                    v = gwork.tile([128, 4, GPC], F32)
                    nc.vector.tensor_add(v[:], gps[:],
                                         xps[:, :, l * GPC:(l + 1) * GPC])
                    sig = gwork.tile([128, 3, GPC], F32)
                    nc.scalar.activation(sig[:], v[:, 0:3, :], AF.Sigmoid)
                    tg = gwork.tile([128, GPC], F32)
                    nc.scalar.activation(tg[:], v[:, 3, :], AF.Sigmoid, scale=2.0)
                    nc.vector.tensor_scalar(tg[:], tg[:], 2.0, -1.0, ALU.mult, ALU.add)
                    t1 = gwork.tile([128, GPC], F32)
                    nc.vector.tensor_mul(t1[:], sig[:, 1, :], cstate[:])
                    t2 = gwork.tile([128, GPC], F32)
                    nc.vector.tensor_mul(t2[:], sig[:, 0, :], tg[:])
                    nc.vector.tensor_add(cstate[:], t1[:], t2[:])
                    tc2 = gwork.tile([128, GPC], F32)
                    nc.scalar.activation(tc2[:], cstate[:], AF.Sigmoid, scale=2.0)
                    nc.vector.tensor_scalar(tc2[:], tc2[:], 2.0, -1.0, ALU.mult, ALU.add)
                    nc.vector.tensor_mul(hstate[:], sig[:, 2, :], tc2[:])
                    nc.scalar.activation(hgT[:, l, :], hstate[:], AF.Relu)

            # ================= attention (per gene) =================
            with tc.tile_pool(name="awork2", bufs=3) as aw, \
                 tc.tile_pool(name="psT", bufs=1, space=bass.MemorySpace.PSUM) as psT:
                for g in range(GPC):
                    hgT_g = hgT[:, :, g]
                    hT_g = hT[:, :, g]
                    tp1 = psT.tile([LC, 128], F32)
                    nc.tensor.transpose(tp1[:], hgT_g, ident[:])
                    hg_s = aw.tile([LC, 128], F32)
                    nc.vector.tensor_copy(hg_s[:], tp1[:])
                    tp2 = psT.tile([LC, 128], F32)
                    nc.tensor.transpose(tp2[:], hT_g, ident[:])
                    h_s = aw.tile([LC, 128], F32)
                    nc.vector.tensor_copy(h_s[:], tp2[:])
                    aT = psT.tile([128, 128], F32)
                    nc.tensor.matmul(aT[:], hg_s[:], h_s[:])
                    nmax = aw.tile([128, 1], F32)
                    nc.vector.tensor_reduce(nmax[:], aT[:], mybir.AxisListType.X,
                                            ALU.max, negate=True)
                    eT = aw.tile([128, 128], F32)
                    rs = aw.tile([128, 1], F32)
                    nc.scalar.activation(eT[:], aT[:], AF.Exp, bias=nmax[:, 0:1],
                                         accum_out=rs[:, 0:1])
                    rc = aw.tile([128, 1], F32)
                    nc.vector.reciprocal(rc[:], rs[:])
                    wps = psT.tile([128, 128], F32)
                    nc.tensor.transpose(wps[:], eT[:], ident[:])
                    w_s = aw.tile([128, 128], F32)
                    nc.vector.tensor_copy(w_s[:], wps[:])
                    bps = psT.tile([LC, 128], F32)
                    nc.tensor.matmul(bps[:], hT_g, w_s[:])
                    b_s = aw.tile([LC, 128], F32)
                    nc.vector.tensor_copy(b_s[:], bps[:])
                    btp = psT.tile([128, LC], F32)
                    nc.tensor.transpose(btp[:], b_s[:], ident[0:LC, 0:LC])
                    nc.scalar.activation(bT[:, :, g], btp[:], AF.Copy,
                                         scale=rc[:, 0:1])

            # ================= all-gather hg/b + top projection =================
            hb_in = dram.tile([2, 128, LC, GPC], F32)
            nc.gpsimd.dma_start(hb_in[0], hgT[:])
            nc.gpsimd.dma_start(hb_in[1], bT[:])
            hb_out = dram.tile([NCORES, 2, 128, LC, GPC], F32)
            nc.gpsimd.collective_compute(
                "AllGather", ALU.bypass, replica_groups=[list(range(NCORES))],
                ins=[hb_in.opt()], outs=[hb_out.opt()])

            with tc.tile_pool(name="big", bufs=1) as big, \
                 tc.tile_pool(name="wstream", bufs=3) as wstream, \
                 tc.tile_pool(name="psX", bufs=1, space=bass.MemorySpace.PSUM) as psX:
                hgF = big.tile([128, LC, B], F32)
                bF = big.tile([128, LC, B], F32)
                for c in range(NCORES):
                    nc.gpsimd.dma_start(hgF[:, :, c * GPC:(c + 1) * GPC],
                                        hb_out[c, 0])
                    nc.gpsimd.dma_start(bF[:, :, c * GPC:(c + 1) * GPC],
                                        hb_out[c, 1])

                xp_ps = psX.tile([128, B], F32)
                for kc in range(158):
                    wt = wstream.tile([128, 128], F32)
                    nc.gpsimd.dma_start(wt[:], D['wihshard'][kc])
                    l, half = kc // 2, kc % 2
                    rhs = (hgF if half == 0 else bF)[:, l, :]
                    nc.tensor.matmul(xp_ps[:], wt[:], rhs,
                                     start=(kc == 0), stop=(kc == 157))
                xp_sb = big.tile([128, B], F32)
                nc.vector.tensor_copy(xp_sb[:], xp_ps[:])
                xp_in = dram.tile([128, B], F32)
                nc.gpsimd.dma_start(xp_in[:], xp_sb[:])
                xp_out = dram.tile([NCORES, 128, B], F32)
                nc.gpsimd.collective_compute(
                    "AllGather", ALU.bypass, replica_groups=[list(range(NCORES))],
                    ins=[xp_in.opt()], outs=[xp_out.opt()])

                # ================= top bi-LSTM scans (both dirs, all cores) ======
                with tc.tile_pool(name="tl", bufs=1) as tl, \
                     tc.tile_pool(name="twork", bufs=4) as tw, \
                     tc.tile_pool(name="psS", bufs=2, space=bass.MemorySpace.PSUM) as psS:
                    xpf = tl.tile([128, 4, B], F32)
                    xpb = tl.tile([128, 4, B], F32)
                    for gate in range(4):
                        nc.gpsimd.dma_start(xpf[:, gate, :], xp_out[gate])
                        nc.gpsimd.dma_start(xpb[:, gate, :], xp_out[4 + gate])
                    biasf = tl.tile([128, 4], F32)
                    nc.gpsimd.dma_start(biasf[:], D['biasf'][:])
                    biasb = tl.tile([128, 4], F32)
                    nc.gpsimd.dma_start(biasb[:], D['biasb'][:])
                    for gate in range(4):
                        nc.vector.tensor_scalar_add(xpf[:, gate, :], xpf[:, gate, :],
                                                    biasf[:, gate:gate + 1])
                        nc.vector.tensor_scalar_add(xpb[:, gate, :], xpb[:, gate, :],
                                                    biasb[:, gate:gate + 1])
                    whhf = tl.tile([128, 512], F32)
                    nc.gpsimd.dma_start(whhf[:], D['whhfT'][:])
                    whhb = tl.tile([128, 512], F32)
                    nc.gpsimd.dma_start(whhb[:], D['whhbT'][:])

                    of_sb = tl.tile([128, B], F32)
                    ob_sb = tl.tile([128, B], F32)
                    zcol = tl.tile([128, 1], F32)
                    nc.vector.memset(zcol[:], 0.0)
                    cf = tl.tile([128, 1], F32); nc.vector.memset(cf[:], 0.0)
                    cb = tl.tile([128, 1], F32); nc.vector.memset(cb[:], 0.0)

                    def lstm_step(whh, xp_t, cstate, hprev, hout):
                        gps = psS.tile([128, 4], F32)
                        for gate in range(4):
                            nc.tensor.matmul(gps[:, gate:gate + 1],
                                             whh[:, bass.ts(gate, 128)], hprev)
                        v = tw.tile([128, 4], F32)
                        nc.vector.tensor_add(v[:], gps[:], xp_t)
                        sig = tw.tile([128, 3], F32)
                        nc.scalar.activation(sig[:], v[:, 0:3], AF.Sigmoid)
                        tg = tw.tile([128, 1], F32)
                        nc.scalar.activation(tg[:], v[:, 3:4], AF.Sigmoid, scale=2.0)
                        nc.vector.tensor_scalar(tg[:], tg[:], 2.0, -1.0,
                                                ALU.mult, ALU.add)
                        t1 = tw.tile([128, 1], F32)
                        nc.vector.tensor_mul(t1[:], sig[:, 1:2], cstate[:])
                        t2 = tw.tile([128, 1], F32)
                        nc.vector.tensor_mul(t2[:], sig[:, 0:1], tg[:])
                        nc.vector.tensor_add(cstate[:], t1[:], t2[:])
                        tc2 = tw.tile([128, 1], F32)
                        nc.scalar.activation(tc2[:], cstate[:], AF.Sigmoid, scale=2.0)
                        nc.vector.tensor_scalar(tc2[:], tc2[:], 2.0, -1.0,
                                                ALU.mult, ALU.add)
                        nc.vector.tensor_mul(hout, sig[:, 2:3], tc2[:])

                    for t in range(B):
                        hpf = zcol[:] if t == 0 else of_sb[:, t - 1:t]
                        lstm_step(whhf, xpf[:, :, t], cf, hpf, of_sb[:, t:t + 1])
                        hpb = zcol[:] if t == 0 else ob_sb[:, B - t:B - t + 1]
                        lstm_step(whhb, xpb[:, :, B - 1 - t], cb, hpb,
                                  ob_sb[:, B - 1 - t:B - t])

                    # ================= FFN + softmax =================
                    w1tA = tl.tile([128, 32], F32)
                    nc.gpsimd.dma_start(w1tA[:], D['w1tA'][:])
                    w1tB = tl.tile([128, 32], F32)
                    nc.gpsimd.dma_start(w1tB[:], D['w1tB'][:])
                    b1col = tl.tile([32, 1], F32)
                    nc.gpsimd.dma_start(b1col[:], D['b1col'][:])
                    w2t = tl.tile([32, 1], F32)
                    nc.gpsimd.dma_start(w2t[:], D['w2t'][:])

                    hhA = tl.tile([128, B], F32)
                    nc.scalar.activation(hhA[:], of_sb[:], AF.Lrelu, alpha=0.01)
                    hhB = tl.tile([128, B], F32)
                    nc.scalar.activation(hhB[:], ob_sb[:], AF.Lrelu, alpha=0.01)
                    z1ps = psS.tile([32, B], F32)
                    nc.tensor.matmul(z1ps[:], w1tA[:], hhA[:], start=True, stop=False)
                    nc.tensor.matmul(z1ps[:], w1tB[:], hhB[:], start=False, stop=True)
                    z1 = tl.tile([32, B], F32)
                    nc.scalar.activation(z1[:], z1ps[:], AF.Lrelu,
                                         bias=b1col[:, 0:1], alpha=0.01)
                    z2ps = psS.tile([1, B], F32)
                    nc.tensor.matmul(z2ps[:], w2t[:], z1[:])
                    nmax = tl.tile([1, 1], F32)
                    nc.vector.tensor_reduce(nmax[:], z2ps[:], mybir.AxisListType.X,
                                            ALU.max, negate=True)
                    ez = tl.tile([1, B], F32)
                    rs = tl.tile([1, 1], F32)
                    nc.scalar.activation(ez[:], z2ps[:], AF.Exp, bias=nmax[:, 0:1],
                                         accum_out=rs[:, 0:1])
                    rc = tl.tile([1, 1], F32)
                    nc.vector.reciprocal(rc[:], rs[:])
                    o_sb = tl.tile([1, B], F32)
                    nc.vector.tensor_scalar(o_sb[:], ez[:], rc[:, 0:1], None, ALU.mult)
                    nc.gpsimd.dma_start(out_d[:], o_sb[:])

    nc.compile()
    return nc


def kernel(x, edge_index, edge_attr, params):
    in_maps, K_c = _prep(x, edge_index, edge_attr, params)
    if K_c not in _cache:
        _cache[K_c] = _build(K_c)
    nc = _cache[K_c]
    res = run_bass_kernel_spmd(nc, in_maps, core_ids=list(range(NCORES)),
                               trace=bool(int(os.environ.get('KTRACE', '0'))))
    kernel.last_result = res
    return res.results[0]['out'].reshape(B).astype(np.float32)
